# revision 31
# baseline (speedup 1.0000x reference)
"""Position Attention Module (DANet) on 8 Trainium2 NeuronCores.

Reference computation (per batch b of 4):
  xf = x[b] : [C=512, N=4096]
  q = Wq@xf + bq : [64, N];  k = Wk@xf + bk : [64, N];  v = Wv@xf + bv : [512, N]
  scores[i,j] = q[:,i].k[:,j];  attn = softmax_j(scores)
  out[c,i] = alpha * sum_j v[c,j] attn[i,j]

Sharding: 2 cores per batch, each core owns half the query rows (i), full k/v.
Per-core x is pre-rolled on host so the owned i-half is always columns 0:2048.

Device design (v3):
  - alpha folded into Wv/bv on the host. Wq/Wk packed into one [C,128]
    stationary so q and k project in a single matmul per (jb, cc).
  - x loaded once as 8 big [128,2048] DMAs (critical-path first: x half0
    before wv / x half1), bitcast f32r; all projections run f32r
    (1 cyc/row at free>=256 vs 4 cyc/row for fp32).
  - scoresT [j, i] as a single K=64 f32r matmul (no hi/lo split): score
    abs err ~6e-3 -> attn rel err ~0.6%, inside the 2e-2 gate.
  - exp without max-subtraction: scores ~ N(0,64); |s|max ~ 40 << 88.
  - exp output and vT stored bf16; AV matmuls run bf16. PSUM accum fp32.
  - softmax denominator: DVE-accumulated exp sums (fp32 bits in an
    f32r-typed tile); ones[128,128] f32r matmul broadcasts the
    partition-sum to all lanes; DVE reciprocal; fused DVE scaling.
  - pso bufs=4 / epool bufs=8 so scores+exp run well ahead of the AV
    chain (measured: each extra lookahead stage removed exp-wait stalls).
"""
import numpy as np


B, C, HW = 4, 512, 4096
CQ = 64
NCORES = 8
IH = HW // 2          # 2048 query rows per core
ITILE = 512           # i-tile (psum free dim)
NITILES = IH // ITILE # 4
JT = 128              # j-tile (contraction chunk for AV / scores lhsT cols)
NJT = HW // JT        # 32
JB = 512              # j-block for projections
XH = 2048             # x half width (one DMA per cc per half)
NCC = C // 128        # 4 contraction chunks of 128 over C

_cache = {}


def _build():
    import concourse.bacc as bacc
    import concourse.tile as tile
    import concourse.mybir as mybir
    from concourse.bass_utils import run_bass_kernel_spmd

    f32 = mybir.dt.float32
    f32r = mybir.dt.float32r
    bf16 = mybir.dt.bfloat16
    AF = mybir.ActivationFunctionType

    nc = bacc.Bacc("TRN2", target_bir_lowering=False, debug=False)

    x_d = nc.dram_tensor("x", [C, HW], f32, kind="ExternalInput")
    wqk_d = nc.dram_tensor("wqk", [C, 128], f32, kind="ExternalInput")
    wvt_d = nc.dram_tensor("wvt", [C, C], f32, kind="ExternalInput")
    bqk_d = nc.dram_tensor("bqk", [128, 1], f32, kind="ExternalInput")
    bv_d = nc.dram_tensor("bv", [1, C], f32, kind="ExternalInput")
    out_d = nc.dram_tensor("out", [C, IH], f32, kind="ExternalOutput")

    with tile.TileContext(nc) as tc:
        with (
            tc.tile_pool(name="const", bufs=1) as cpool,
            tc.tile_pool(name="kq", bufs=1) as kqpool,
            tc.tile_pool(name="vt", bufs=1) as vtpool,
            tc.tile_pool(name="xin", bufs=8) as xpool,
        ):
            # --- DMA issue order (each issue slice costs ~630ns on Sync):
            # wqk -> x jb0 -> wvt -> biases -> x jb1..3 -> x half1, so the
            # first kq chain (~11us) and first v chain (~13us) unblock early
            wqk = [cpool.tile([128, 128], f32r, tag=f"wqk{i}", name=f"wqk{i}") for i in range(NCC)]
            wvt = [cpool.tile([128, C], f32r, tag=f"wvt{i}", name=f"wvt{i}") for i in range(NCC)]
            for cc in range(NCC):
                sl = slice(cc * 128, (cc + 1) * 128)
                nc.sync.dma_start(wqk[cc][:], wqk_d[sl, :].bitcast(f32r))
            xt = [[None] * NCC for _ in range(2)]
            for cc in range(NCC):
                t = xpool.tile([128, XH], f32r, tag="x", name=f"x0_{cc}")
                xt[0][cc] = t
            for cc in range(NCC):
                csl = slice(cc * 128, (cc + 1) * 128)
                nc.sync.dma_start(xt[0][cc][:, 0:JB], x_d[csl, 0:JB].bitcast(f32r))
            for cc in range(NCC):
                sl = slice(cc * 128, (cc + 1) * 128)
                nc.sync.dma_start(wvt[cc][:], wvt_d[sl, :].bitcast(f32r))
            bqk_c = cpool.tile([128, 1], f32, tag="bqkc")
            nc.sync.dma_start(bqk_c[:], bqk_d[:])
            bv_row = cpool.tile([1, C], f32r, tag="bvrow")
            nc.sync.dma_start(bv_row[:], bv_d[:].bitcast(f32r))
            for jb in range(1, XH // JB):
                jsl = slice(jb * JB, (jb + 1) * JB)
                for cc in range(NCC):
                    csl = slice(cc * 128, (cc + 1) * 128)
                    nc.sync.dma_start(xt[0][cc][:, jsl], x_d[csl, jsl].bitcast(f32r))
            for cc in range(NCC):
                csl = slice(cc * 128, (cc + 1) * 128)
                t = xpool.tile([128, XH], f32r, tag="x", name=f"x1_{cc}")
                nc.sync.dma_start(t[:], x_d[csl, XH:HW].bitcast(f32r))
                xt[1][cc] = t

            ones_r = cpool.tile([1, 128], f32, tag="onesr")    # K=1 bcast lhsT
            nc.vector.memset(ones_r[:], 1.0)
            ones_sq = cpool.tile([128, 128], f32, tag="onessq")  # sum+bcast lhsT
            nc.vector.memset(ones_sq[:], 1.0)

            # k/q activations for scores, f32r single precision
            KHL = kqpool.tile([CQ, HW], f32r, tag="khl")
            QH = kqpool.tile([CQ, IH], f32r, tag="qh")
            vts = [vtpool.tile([JT, C], bf16, tag=f"vt{j}", name=f"vt{j}") for j in range(NJT)]

            # bvB: (alpha*bv) broadcast to 128 partitions (for vT psum eviction)
            with tc.tile_pool(name="ppre", bufs=1, space="PSUM") as ppre:
                bvB = cpool.tile([128, C], f32, tag="bvB")
                ps = ppre.tile([128, C], f32, tag="bvps")
                nc.tensor.matmul(ps[:], ones_r[:].bitcast(f32r), bv_row[:], start=True, stop=True)
                nc.vector.tensor_copy(bvB[:], ps[:])

            # ---------------- projections ----------------
            with (
                tc.tile_pool(name="pkq", bufs=2, space="PSUM") as pkq,
                tc.tile_pool(name="pvt", bufs=3, space="PSUM") as pvt,
            ):
                for half in range(2):
                    for jb in range(HW // 2 // JB):
                        lsl = slice(jb * JB, (jb + 1) * JB)       # within x tile
                        gof = half * XH + jb * JB                  # global j offset
                        gsl = slice(gof, gof + JB)
                        if half == 0:
                            # packed q(rows 0:64) + k(rows 64:128) projection
                            kqp = pkq.tile([128, JB], f32, tag="kqp")
                            for cc in range(NCC):
                                nc.tensor.matmul(kqp[:], wqk[cc][:], xt[half][cc][:, lsl],
                                                 start=(cc == 0), stop=(cc == NCC - 1))
                            nc.scalar.activation(QH[:, gsl], kqp[0:CQ, :], AF.Identity,
                                                 bias=bqk_c[0:CQ])
                            nc.scalar.activation(KHL[:, gsl], kqp[CQ:128, :], AF.Identity,
                                                 bias=bqk_c[CQ:128])
                        else:
                            kp = pkq.tile([CQ, JB], f32, tag="kqp")
                            for cc in range(NCC):
                                nc.tensor.matmul(kp[:], wqk[cc][:, CQ:128], xt[half][cc][:, lsl],
                                                 start=(cc == 0), stop=(cc == NCC - 1))
                            nc.scalar.activation(KHL[:, gsl], kp[:], AF.Identity,
                                                 bias=bqk_c[CQ:128])
                        # vT tiles [128 j, C] in bf16
                        for js in range(JB // JT):
                            vp = pvt.tile([JT, C], f32, tag="vtp")
                            for cc in range(NCC):
                                nc.tensor.matmul(
                                    vp[:], xt[half][cc][:, jb * JB + js * JT:jb * JB + (js + 1) * JT],
                                    wvt[cc][:], start=(cc == 0), stop=(cc == NCC - 1))
                            nc.vector.tensor_add(vts[(gof // JT) + js][:], vp[:], bvB[:])

            # ---------------- attention ----------------
            with (
                tc.tile_pool(name="expp", bufs=10) as epool,
                tc.tile_pool(name="dnm", bufs=2) as dpool,
                tc.tile_pool(name="ost", bufs=8) as opool,
                tc.tile_pool(name="rows", bufs=2) as rpool,
                tc.tile_pool(name="pso", bufs=4, space="PSUM") as pso,
                tc.tile_pool(name="pout", bufs=4, space="PSUM") as pout,
            ):
                for it in range(NITILES):
                    isl = slice(it * ITILE, (it + 1) * ITILE)
                    ops = [pout.tile([128, ITILE], f32, tag="op", name=f"op{it}_{i}") for i in range(NCC)]
                    dnm = dpool.tile([128, ITILE], f32r, tag="dn")
                    for j in range(NJT):
                        jsl = slice(j * JT, (j + 1) * JT)
                        sp = pso.tile([JT, ITILE], f32, tag="sc")
                        nc.tensor.matmul(sp[:], KHL[:, jsl], QH[:, isl],
                                         start=True, stop=True)
                        et = epool.tile([JT, ITILE], bf16, tag="exp")
                        nc.scalar.activation(et[:], sp[:], AF.Exp)
                        if j == 0:
                            nc.vector.tensor_copy(dnm[:], et[:])
                        else:
                            nc.vector.tensor_add(dnm[:], dnm[:], et[:])
                        for cc in range(NCC):
                            nc.tensor.matmul(
                                ops[cc][:], vts[j][:, cc * 128:(cc + 1) * 128], et[:],
                                start=(j == 0), stop=(j == NJT - 1))
                    # denomB = column-sums of dnm broadcast to all 128 partitions
                    dB = pso.tile([128, ITILE], f32, tag="sc")
                    nc.tensor.matmul(dB[:], ones_sq[:].bitcast(f32r), dnm[:], start=True, stop=True)
                    recipB = rpool.tile([128, ITILE], f32, tag="recipB")
                    nc.vector.reciprocal_approx_fast(out=recipB[:], in_=dB[:])
                    for cc in range(NCC):
                        ot = opool.tile([128, ITILE], f32, tag="ot")
                        nc.vector.tensor_mul(ot[:], ops[cc][:], recipB[:])
                        nc.sync.dma_start(out_d[cc * 128:(cc + 1) * 128, isl], ot[:])

    nc.compile()
    return nc, run_bass_kernel_spmd


def kernel(x, Wq, bq, Wk, bk, Wv, bv, alpha, trace=False, trace_kwargs=None):
    if "nc" not in _cache:
        _cache["nc"] = _build()
    nc, run_spmd = _cache["nc"]

    x = np.ascontiguousarray(np.asarray(x, dtype=np.float32)).reshape(B, C, HW)
    a = float(np.asarray(alpha, np.float32).reshape(-1)[0])
    wqk = np.ascontiguousarray(
        np.concatenate([np.asarray(Wq, np.float32).T, np.asarray(Wk, np.float32).T], axis=1))
    wvt = np.ascontiguousarray(np.asarray(Wv, np.float32).T * a)
    bqk = np.concatenate([np.asarray(bq, np.float32).reshape(CQ),
                          np.asarray(bk, np.float32).reshape(CQ)]).reshape(128, 1)
    bv = (np.asarray(bv, np.float32) * a).reshape(1, C)

    in_maps = []
    for core in range(NCORES):
        b, ih = core // 2, core % 2
        xb = x[b]
        if ih:
            xb = np.ascontiguousarray(np.concatenate([xb[:, IH:], xb[:, :IH]], axis=1))
        in_maps.append({"x": xb, "wqk": wqk, "wvt": wvt, "bqk": bqk, "bv": bv})

    kwargs = {}
    if trace:
        kwargs["trace"] = True
        kwargs.update(trace_kwargs or {})
    res = run_spmd(nc, in_maps, list(range(NCORES)), **kwargs)

    out = np.empty((B, C, HW), dtype=np.float32)
    for core in range(NCORES):
        b, ih = core // 2, core % 2
        out[b][:, ih * IH:(ih + 1) * IH] = res.results[core]["out"]
    if trace:
        return out.reshape(B, C, 64, 64), res
    return out.reshape(B, C, 64, 64)
